# revision 44
# baseline (speedup 1.0000x reference)
"""DFT spectrogram (nn_DftSpectrogram) Bass kernel for 8 Trainium2 NeuronCores.

Pure data parallel: 32 batch items -> 4 per core. Per item (T=96512 samples):
  - 601 frames of 512 taps (stride 160) are loaded as 5 overlapping blocks of
    128 frames in [frame, tap] layout (contiguous 2KB rows -> efficient DMA)
  - folded with the DFT basis symmetry about tap 256: u[j]=x[j]+x[512-j]
    (cos side), v[j]=x[j]-x[512-j] (sin side) halve the matmul contraction to
    256; the j=0 term is folded into the ACT Square bias; the j=256 basis row
    is halved on the host to absorb the self-pairing
  - the whole PE path runs in fp16: folds emit fp16 u/v, PE-transposes move
    fp16 (~110ns vs fp32's 358ns, measured on silicon), matmuls contract fp16
    against fp16 bases with fp32 PSUM accumulation (107.6ns vs fp32's 413ns
    per [128c x 256free] matmul, measured). fp16's 11-bit mantissa keeps the
    end-to-end l2 error ~1e-3 vs the 2e-2 gate.
  - log-magnitude + per-frame mean/std normalization over the 256 freqs is
    done in [frame, k] layout in fp32 exactly as before
  - the normalized [frame, k] tiles are DMA'd straight out; the host fixes the
    layout to [k, frame] while gathering the 8 core shards

Engine balance per block: PE does 4 transposes + 4 fp16 matmuls; u-fold on
DVE, v-fold and r^2+i^2 on GPSIMD (otherwise idle), both squares (fused
PSUM->SBUF move) and ln on ACT, bn_stats on DVE, the PSUM->SBUF framesT copy
alternates ACT/DVE by block parity.

1/(sqrt(var)+eps') is computed entirely on DVE (int bit-trick seed + two
Heron steps) so ACT only ever needs the natural_log table set. eps
compensation keeps the algebra exact:
(fft-mean)/(std+1e-7) == (g-mean_g)/(std_g+2*ln(10)*1e-7).
"""
from contextlib import ExitStack

import numpy as np

import concourse.bass as bass
import concourse.tile as tile
from concourse import bacc, mybir
from concourse.bass_utils import run_bass_kernel_spmd

N_CORES = 8
B_FULL = 32
C_FULL = 1
T = 96512
NFFT = 512
KOUT = 256          # only lower half of the spectrum is kept
SHIFT = 160
F = (T - NFFT) // SHIFT + 1  # 601
BPC = B_FULL // N_CORES      # 4 items per core
EPS = 1e-7
CEPS = float(2.0 * np.log(10.0) * 1e-7)
F0S = (0, 128, 256, 384, 473)  # frame-block starts; last block overlaps by 39
FP32 = mybir.dt.float32
FP16 = mybir.dt.float16
MM_DT = FP16  # fp16 PE path: 1 cycle/row matmuls + fast transposes


def _build(ctx: ExitStack, tc: "tile.TileContext", xh, wrh, wih, idh, id32h,
           outh, mm_dt, reps: int):
    nc = tc.nc
    AP = bass.AP
    AF = mybir.ActivationFunctionType

    consts = ctx.enter_context(tc.tile_pool(name="consts", bufs=1))
    fpool = ctx.enter_context(tc.tile_pool(name="frames", bufs=1))
    uvpool = ctx.enter_context(tc.tile_pool(name="uv", bufs=4))
    ftpool = ctx.enter_context(tc.tile_pool(name="framesT", bufs=4))
    mpool = ctx.enter_context(tc.tile_pool(name="mag", bufs=8))
    glpool = ctx.enter_context(tc.tile_pool(name="gl", bufs=4))
    spool = ctx.enter_context(tc.tile_pool(name="stats", bufs=6))
    gnpool = ctx.enter_context(tc.tile_pool(name="gnorm", bufs=4))
    ptrp = ctx.enter_context(tc.tile_pool(name="ptr", bufs=2, space="PSUM"))
    ptrvp = ctx.enter_context(tc.tile_pool(name="ptrv", bufs=1, space="PSUM"))
    prip = ctx.enter_context(tc.tile_pool(name="pri", bufs=2, space="PSUM"))
    prip1 = ctx.enter_context(tc.tile_pool(name="pri1", bufs=1, space="PSUM"))

    c_sb = consts.tile([128, 2 * KOUT], mm_dt, tag="c_sb")
    s_sb = consts.tile([128, 2 * KOUT], mm_dt, tag="s_sb")
    ident = consts.tile([128, 128], mm_dt, tag="ident")
    ident32 = consts.tile([128, 128], FP32, tag="ident32")
    epsb = consts.tile([128, 1], FP32, tag="epsb")
    nc.vector.memset(epsb[:], EPS)

    def stage_front(b, ftile, fb, gl, mv3, pri, loc, sqr):
        """u/v fold, transposes, matmuls, biased real square for one block;
        the imag squares/mag2/ln run pair-merged in stage_mid."""
        src_f = ftile[:, fb * NFFT:(fb + 1) * NFFT]
        # fold: u[j] = x[j] + x[512-j], v[j] = x[j] - x[512-j], j=1..256
        fwd = src_f[:, 1:257]
        rev = src_f[:, 511:255:-1]
        # u-fold on DVE converts to fp16 cheaply; GPSIMD's fp16 write path is
        # 1.8x slower than fp32 (measured), so the v-fold stays fp32 and its
        # transposes run fp32 (PE has slack) - the ft copy does the cast
        u = uvpool.tile([128, KOUT], mm_dt, tag="u", name="u")
        nc.vector.tensor_add(u[:], fwd, rev)
        v = uvpool.tile([128, KOUT], FP32, tag="v", name="v")
        nc.gpsimd.tensor_sub(v[:], fwd, rev)

        # transpose u,v [128f, 256j] -> 4x [128j, 128f]
        ptru = ptrp.tile([128, KOUT], mm_dt, tag="ptru", name="ptru")
        for c, srcc in enumerate((u[:, 0:128], u[:, 128:256])):
            nc.tensor.matmul(ptru[:, c * 128:(c + 1) * 128],
                             srcc, ident[:], is_transpose=True,
                             start=(c == 0), stop=(c == 1))
        ptrv = ptrvp.tile([128, KOUT], FP32, tag="ptrv", name="ptrv")
        for c, srcc in enumerate((v[:, 0:128], v[:, 128:256])):
            nc.tensor.matmul(ptrv[:, c * 128:(c + 1) * 128],
                             srcc, ident32[:], is_transpose=True,
                             start=(c == 0), stop=(c == 1))
        ft_sb = ftpool.tile([128, NFFT], mm_dt, tag="ft_sb", name="ft_sb")
        nc.vector.tensor_copy(ft_sb[:, 0:KOUT], ptru[:])
        nc.scalar.copy(ft_sb[:, KOUT:2 * KOUT], ptrv[:])

        # real[f,k] = sum_j u[f,j] C[j,k] (+ x[160f], via Square bias)
        # imag[f,k] = sum_j v[f,j] S[j,k]
        o = loc * NFFT
        nc.tensor.matmul(pri[:, o:o + KOUT], ft_sb[:, 0:128],
                         c_sb[:, 0:KOUT], start=True, stop=False)
        nc.tensor.matmul(pri[:, o:o + KOUT], ft_sb[:, 128:256],
                         c_sb[:, KOUT:2 * KOUT], start=False, stop=False)
        nc.tensor.matmul(pri[:, o + KOUT:o + 2 * KOUT], ft_sb[:, 256:384],
                         s_sb[:, 0:KOUT], start=False, stop=False)
        nc.tensor.matmul(pri[:, o + KOUT:o + 2 * KOUT], ft_sb[:, 384:512],
                         s_sb[:, KOUT:2 * KOUT], start=False, stop=True)
        # biased real square (bias x0 differs per block, so per-block)
        nc.scalar.activation(sqr[:, loc * KOUT:(loc + 1) * KOUT],
                             pri[:, o:o + KOUT], AF.Square,
                             bias=src_f[:, 0:1])

    def stage_mid(b, fbs, gl, mv3, pri, sqr):
        """Pair-merged imag squares, mag2, ln; per-block bn stats."""
        nb = len(fbs)
        fb0 = fbs[0]
        a = pri[:]
        sqi = mpool.tile([128, nb * KOUT], FP32, tag=f"sqi{nb}", name="sqi")
        nc.scalar.activation(
            bass.AP(sqi[:].tensor, sqi[:].offset,
                    [list(sqi[:].ap[0]), [KOUT, nb], [1, KOUT]]),
            bass.AP(a.tensor, a.offset + KOUT,
                    [list(a.ap[0]), [NFFT, nb], [1, KOUT]]),
            AF.Square)
        msum = mpool.tile([128, nb * KOUT], FP32, tag=f"ms{nb}", name="ms")
        for loc in range(nb):
            nc.gpsimd.tensor_add(msum[:, loc * KOUT:(loc + 1) * KOUT],
                                 sqr[:, loc * KOUT:(loc + 1) * KOUT],
                                 sqi[:, loc * KOUT:(loc + 1) * KOUT])
        nc.scalar.activation(gl[:, fb0 * KOUT:(fb0 + nb) * KOUT],
                             msum[:], AF.Ln, bias=epsb[:])
        for loc, fb in enumerate(fbs):
            bn6 = spool.tile([128, 6], FP32, tag="bn6", name="bn6")
            nc.vector.bn_stats(bn6[:], gl[:, fb * KOUT:(fb + 1) * KOUT])
            nc.vector.bn_aggr(mv3[:, fb, :], bn6[:])

    def stage_back(b, gl, mv, fb_lo=0, fb_hi=4):
        """rden = 1/(sqrt(var)+ceps) on DVE only (int bit-trick sqrt seed +
        two Heron steps, 5e-7 rel; keeps ACT on one table set), then
        normalize and DMA out, for blocks fb_lo..fb_hi of one item."""
        w = fb_hi - fb_lo + 1
        var = bass.AP(mv[:].tensor, mv[:].offset + 2 * fb_lo + 1,
                      [list(mv[:].ap[0]), [2, w]])
        sh = spool.tile([128, w], mybir.dt.int32, tag="sh", name="sh")
        nc.vector.tensor_scalar(sh[:], var.bitcast(mybir.dt.int32), 1, None,
                                op0=mybir.AluOpType.arith_shift_right)
        s0i = spool.tile([128, w], mybir.dt.int32, tag="s0i", name="s0i")
        nc.vector.tensor_scalar(s0i[:], sh[:], 0x1FBD1DF5, None,
                                op0=mybir.AluOpType.add)
        s_cur = s0i[:].bitcast(FP32)
        for it in range(2):
            hr = spool.tile([128, w], FP32, tag=f"hr{it}", name=f"hr{it}")
            nc.vector.reciprocal(hr[:], s_cur)
            ht = spool.tile([128, w], FP32, tag=f"ht{it}", name=f"ht{it}")
            nc.vector.tensor_mul(ht[:], var, hr[:])
            hs = spool.tile([128, w], FP32, tag=f"hs{it}", name=f"hs{it}")
            nc.vector.tensor_add(hs[:], s_cur, ht[:])
            if it == 0:
                hh = spool.tile([128, w], FP32, tag="hh0", name="hh0")
                nc.vector.tensor_scalar_mul(hh[:], hs[:], 0.5)
                s_cur = hh[:]
            else:
                s_cur = hs[:]  # final 0.5 folds into the eps-add below
        uu = spool.tile([128, w], FP32, tag="uu", name="uu")
        nc.vector.tensor_scalar(uu[:], s_cur, 0.5, CEPS,
                                op0=mybir.AluOpType.mult,
                                op1=mybir.AluOpType.add)
        rden = spool.tile([128, w], FP32, tag="rden", name="rden")
        nc.vector.reciprocal(rden[:], uu[:])

        # normalize into one tile so blocks 0..3 leave in a single strided
        # DMA; block 4 (only 89 new frames) goes separately
        gn4 = None
        for fb in range(fb_lo, fb_hi + 1):
            gls = gl[:, fb * KOUT:(fb + 1) * KOUT]
            if fb < 4:
                if gn4 is None:
                    gn4 = gnpool.tile([128, 4 * KOUT], FP32, tag="gn4",
                                      name="gn4")
                gdst = gn4[:, fb * KOUT:(fb + 1) * KOUT]
            else:
                gdst = gnpool.tile([128, KOUT], FP32, tag="gn", name="gn")
            nc.vector.tensor_scalar(gdst, gls,
                                    mv[:, 2 * fb:2 * fb + 1],
                                    rden[:, fb - fb_lo:fb - fb_lo + 1],
                                    op0=mybir.AluOpType.subtract,
                                    op1=mybir.AluOpType.mult)
            if fb == 4:
                # frames 473..511 were already written by block 3
                nc.sync.dma_start(outh.ap()[b, 512:601, :], gdst[39:128, :])
        if gn4 is not None:
            nblk = min(fb_hi, 3) - fb_lo + 1
            dst = bass.AP(outh, b * F * KOUT + fb_lo * 128 * KOUT,
                          [[KOUT, 128], [128 * KOUT, nblk], [1, KOUT]])
            nc.sync.dma_start(
                dst, gn4[:, fb_lo * KOUT:(fb_lo + nblk) * KOUT].rearrange(
                    "p (f k) -> p f k", k=KOUT))

    def body():
        # issue every input DMA first: otherwise item b+1's loads queue on
        # the SP HWDGE ring behind item b's dep-gated output DMAs.
        ftiles = [fpool.tile([128, 5 * NFFT], FP32, tag=f"ftile{b}",
                             name=f"ftile{b}") for b in range(BPC)]

        def fdma(b, fb):
            srcb = AP(xh, b * T + SHIFT * F0S[fb], [[SHIFT, 128], [1, NFFT]])
            nc.sync.dma_start(ftiles[b][:, fb * NFFT:(fb + 1) * NFFT], srcb)

        fdma(0, 0)
        if reps == 1:
            nc.sync.dma_start(ident[:], idh.ap())
            nc.sync.dma_start(ident32[:], id32h.ap())
            nc.sync.dma_start(c_sb[:].rearrange("p (c k) -> p c k", k=KOUT),
                              wrh.ap().rearrange("(c p) k -> p c k", p=128))
            nc.sync.dma_start(s_sb[:].rearrange("p (c k) -> p c k", k=KOUT),
                              wih.ap().rearrange("(c p) k -> p c k", p=128))
        fdma(0, 1)
        for b in range(BPC):
            for fb in range(5):
                if (b, fb) not in ((0, 0), (0, 1)):
                    fdma(b, fb)

        pending = None
        for b in range(BPC):
            last = b == BPC - 1
            gl = glpool.tile([128, 5 * KOUT], FP32, tag="gl", name="gl")
            mv = spool.tile([128, 10], FP32, tag="mv", name="mv")
            mv3 = mv[:].rearrange("p (f two) -> p f two", two=2)
            for fbs in ((0, 1), (2, 3), (4,)):
                nb = len(fbs)
                pp = prip if nb == 2 else prip1
                pri = pp.tile([128, nb * NFFT], FP32, tag=f"pri{nb}",
                              name="pri")
                sqr = mpool.tile([128, nb * KOUT], FP32, tag=f"sqr{nb}",
                                 name="sqr")
                for loc, fb in enumerate(fbs):
                    stage_front(b, ftiles[b], fb, gl, mv3, pri, loc, sqr)
                stage_mid(b, fbs, gl, mv3, pri, sqr)
                if fbs[0] == 2 and pending is not None:
                    stage_back(*pending)
                    pending = None
                if last and fbs[0] == 2:
                    # split the final item's normalization so only block 4's
                    # short chain sits in the kernel tail
                    stage_back(b, gl, mv, 0, 3)
            if not last:
                pending = (b, gl, mv)
            else:
                stage_back(b, gl, mv, 4, 4)

    if reps == 1:
        body()
    else:
        nc.sync.dma_start(ident[:], idh.ap())
        nc.sync.dma_start(ident32[:], id32h.ap())
        nc.sync.dma_start(c_sb[:].rearrange("p (c k) -> p c k", k=KOUT),
                          wrh.ap().rearrange("(c p) k -> p c k", p=128))
        nc.sync.dma_start(s_sb[:].rearrange("p (c k) -> p c k", k=KOUT),
                          wih.ap().rearrange("(c p) k -> p c k", p=128))
        with tc.For_i(0, reps, 1):
            body()


def build_nc(mm_dt=MM_DT, reps: int = 1):
    nc = bacc.Bacc("TRN2", target_bir_lowering=False, debug=False)
    xh = nc.dram_tensor("x", [BPC, T], FP32, kind="ExternalInput")
    wrh = nc.dram_tensor("wr", [KOUT, KOUT], mm_dt, kind="ExternalInput")
    wih = nc.dram_tensor("wi", [KOUT, KOUT], mm_dt, kind="ExternalInput")
    idh = nc.dram_tensor("ident", [128, 128], mm_dt, kind="ExternalInput")
    id32h = nc.dram_tensor("ident32", [128, 128], FP32, kind="ExternalInput")
    outh = nc.dram_tensor("out", [BPC, F, KOUT], FP32, kind="ExternalOutput")
    with tile.TileContext(nc) as tc, ExitStack() as ctx:
        _build(ctx, tc, xh, wrh, wih, idh, id32h, outh, mm_dt, reps)
    nc.compile()
    return nc


def make_in_maps(x, W_real, W_imag):
    xs = np.asarray(x, dtype=np.float32).reshape(B_FULL, T)
    Wr = np.asarray(W_real, np.float32)
    Wi = np.asarray(W_imag, np.float32)
    # folded bases, rows j=1..256; j=256 halved (cos) / zero (sin, exact)
    wr_dev = np.zeros((KOUT, KOUT), np.float32)
    wi_dev = np.zeros((KOUT, KOUT), np.float32)
    wr_dev[:255] = Wr[:KOUT, 1:256].T
    wr_dev[255] = 0.5 * Wr[:KOUT, 256]
    wi_dev[:255] = Wi[:KOUT, 1:256].T
    wi_dev[255] = 0.0
    hdt = np.float16 if MM_DT == FP16 else np.float32
    ident = np.eye(128, dtype=hdt)
    return [
        {"x": np.ascontiguousarray(xs[i * BPC:(i + 1) * BPC]),
         "wr": wr_dev.astype(hdt), "wi": wi_dev.astype(hdt), "ident": ident,
         "ident32": np.eye(128, dtype=np.float32)}
        for i in range(N_CORES)
    ]


_NC_CACHE = {}


def kernel(x, W_real, W_imag):
    key = (str(MM_DT), 1)
    if key not in _NC_CACHE:
        _NC_CACHE[key] = build_nc(MM_DT, 1)
    nc = _NC_CACHE[key]
    in_maps = make_in_maps(x, W_real, W_imag)
    res = run_bass_kernel_spmd(nc, in_maps, core_ids=list(range(N_CORES)))
    out = np.concatenate([np.asarray(r["out"]) for r in res.results], axis=0)
    out = np.ascontiguousarray(out.transpose(0, 2, 1))             # [32, K, F]
    return out.reshape(B_FULL, C_FULL, KOUT, F).astype(np.float32)


# revision 45
# speedup vs baseline: 1.7454x; 1.7454x over previous
"""DFT spectrogram (nn_DftSpectrogram) Bass kernel for 8 Trainium2 NeuronCores.

Pure data parallel: 32 batch items -> 4 per core. Per item (T=96512 samples):
  - 601 frames of 512 taps (stride 160) are loaded as 5 overlapping blocks of
    128 frames in [frame, tap] layout (contiguous 2KB rows -> efficient DMA)
  - folded with the DFT basis symmetry about tap 256: u[j]=x[j]+x[512-j]
    (cos side), v[j]=x[j]-x[512-j] (sin side) halve the matmul contraction to
    256; the j=0 term is folded into the ACT Square bias; the j=256 basis row
    is halved on the host to absorb the self-pairing
  - the whole PE path runs in fp16: folds emit fp16 u/v, PE-transposes move
    fp16 (~110ns vs fp32's 358ns, measured on silicon), matmuls contract fp16
    against fp16 bases with fp32 PSUM accumulation (107.6ns vs fp32's 413ns
    per [128c x 256free] matmul, measured). fp16's 11-bit mantissa keeps the
    end-to-end l2 error ~1e-3 vs the 2e-2 gate.
  - log-magnitude + per-frame mean/std normalization over the 256 freqs is
    done in [frame, k] layout in fp32 exactly as before
  - the normalized [frame, k] tiles are DMA'd straight out; the host fixes the
    layout to [k, frame] while gathering the 8 core shards

Engine balance per block: PE does 4 transposes + 4 fp16 matmuls; u-fold on
DVE, v-fold and r^2+i^2 on GPSIMD (otherwise idle), both squares (fused
PSUM->SBUF move) and ln on ACT, bn_stats on DVE, the PSUM->SBUF framesT copy
alternates ACT/DVE by block parity.

1/(sqrt(var)+eps') is computed entirely on DVE (int bit-trick seed + two
Heron steps) so ACT only ever needs the natural_log table set. eps
compensation keeps the algebra exact:
(fft-mean)/(std+1e-7) == (g-mean_g)/(std_g+2*ln(10)*1e-7).
"""
from contextlib import ExitStack

import numpy as np

import concourse.bass as bass
import concourse.tile as tile
from concourse import bacc, mybir
from concourse.bass_utils import run_bass_kernel_spmd

N_CORES = 8
B_FULL = 32
C_FULL = 1
T = 96512
NFFT = 512
KOUT = 256          # only lower half of the spectrum is kept
SHIFT = 160
F = (T - NFFT) // SHIFT + 1  # 601
BPC = B_FULL // N_CORES      # 4 items per core
EPS = 1e-7
CEPS = float(2.0 * np.log(10.0) * 1e-7)
F0S = (0, 128, 256, 384, 473)  # frame-block starts; last block overlaps by 39
FP32 = mybir.dt.float32
FP16 = mybir.dt.float16
MM_DT = FP16  # fp16 PE path: 1 cycle/row matmuls + fast transposes


def _build(ctx: ExitStack, tc: "tile.TileContext", xh, wrh, wih, idh, id32h,
           outh, mm_dt, reps: int):
    nc = tc.nc
    AP = bass.AP
    AF = mybir.ActivationFunctionType

    consts = ctx.enter_context(tc.tile_pool(name="consts", bufs=1))
    fpool = ctx.enter_context(tc.tile_pool(name="frames", bufs=1))
    uvpool = ctx.enter_context(tc.tile_pool(name="uv", bufs=4))
    ftpool = ctx.enter_context(tc.tile_pool(name="framesT", bufs=4))
    mpool = ctx.enter_context(tc.tile_pool(name="mag", bufs=8))
    glpool = ctx.enter_context(tc.tile_pool(name="gl", bufs=4))
    spool = ctx.enter_context(tc.tile_pool(name="stats", bufs=6))
    gnpool = ctx.enter_context(tc.tile_pool(name="gnorm", bufs=4))
    ptrp = ctx.enter_context(tc.tile_pool(name="ptr", bufs=2, space="PSUM"))
    ptrvp = ctx.enter_context(tc.tile_pool(name="ptrv", bufs=1, space="PSUM"))
    prip = ctx.enter_context(tc.tile_pool(name="pri", bufs=2, space="PSUM"))
    prip1 = ctx.enter_context(tc.tile_pool(name="pri1", bufs=1, space="PSUM"))

    c_sb = consts.tile([128, 2 * KOUT], mm_dt, tag="c_sb")
    s_sb = consts.tile([128, 2 * KOUT], mm_dt, tag="s_sb")
    ident = consts.tile([128, 128], mm_dt, tag="ident")
    ident32 = consts.tile([128, 128], FP32, tag="ident32")
    epsb = consts.tile([128, 1], FP32, tag="epsb")
    nc.vector.memset(epsb[:], EPS)

    def stage_front(b, ftile, fb, gl, mv3, pri, loc, sqr):
        """u/v fold, transposes, matmuls, biased real square for one block;
        the imag squares/mag2/ln run pair-merged in stage_mid."""
        src_f = ftile[:, fb * NFFT:(fb + 1) * NFFT]
        # fold: u[j] = x[j] + x[512-j], v[j] = x[j] - x[512-j], j=1..256
        fwd = src_f[:, 1:257]
        rev = src_f[:, 511:255:-1]
        # u-fold on DVE converts to fp16 cheaply; GPSIMD's fp16 write path is
        # 1.8x slower than fp32 (measured), so the v-fold stays fp32 and its
        # transposes run fp32 (PE has slack) - the ft copy does the cast
        u = uvpool.tile([128, KOUT], mm_dt, tag="u", name="u")
        nc.vector.tensor_add(u[:], fwd, rev)
        v = uvpool.tile([128, KOUT], FP32, tag="v", name="v")
        nc.gpsimd.tensor_sub(v[:], fwd, rev)

        # transpose u,v [128f, 256j] -> 4x [128j, 128f]
        ptru = ptrp.tile([128, KOUT], mm_dt, tag="ptru", name="ptru")
        for c, srcc in enumerate((u[:, 0:128], u[:, 128:256])):
            nc.tensor.matmul(ptru[:, c * 128:(c + 1) * 128],
                             srcc, ident[:], is_transpose=True,
                             start=(c == 0), stop=(c == 1))
        ptrv = ptrvp.tile([128, KOUT], FP32, tag="ptrv", name="ptrv")
        for c, srcc in enumerate((v[:, 0:128], v[:, 128:256])):
            nc.tensor.matmul(ptrv[:, c * 128:(c + 1) * 128],
                             srcc, ident32[:], is_transpose=True,
                             start=(c == 0), stop=(c == 1))
        ft_sb = ftpool.tile([128, NFFT], mm_dt, tag="ft_sb", name="ft_sb")
        nc.scalar.copy(ft_sb[:, 0:KOUT], ptru[:])
        nc.vector.tensor_copy(ft_sb[:, KOUT:2 * KOUT], ptrv[:])

        # real[f,k] = sum_j u[f,j] C[j,k] (+ x[160f], via Square bias)
        # imag[f,k] = sum_j v[f,j] S[j,k]
        o = loc * NFFT
        nc.tensor.matmul(pri[:, o:o + KOUT], ft_sb[:, 0:128],
                         c_sb[:, 0:KOUT], start=True, stop=False)
        nc.tensor.matmul(pri[:, o:o + KOUT], ft_sb[:, 128:256],
                         c_sb[:, KOUT:2 * KOUT], start=False, stop=False)
        nc.tensor.matmul(pri[:, o + KOUT:o + 2 * KOUT], ft_sb[:, 256:384],
                         s_sb[:, 0:KOUT], start=False, stop=False)
        nc.tensor.matmul(pri[:, o + KOUT:o + 2 * KOUT], ft_sb[:, 384:512],
                         s_sb[:, KOUT:2 * KOUT], start=False, stop=True)
        # biased real square (bias x0 differs per block, so per-block)
        nc.scalar.activation(sqr[:, loc * KOUT:(loc + 1) * KOUT],
                             pri[:, o:o + KOUT], AF.Square,
                             bias=src_f[:, 0:1])

    def stage_mid(b, fbs, gl, mv3, pri, sqr):
        """Pair-merged imag squares, mag2, ln; per-block bn stats."""
        nb = len(fbs)
        fb0 = fbs[0]
        a = pri[:]
        sqi = mpool.tile([128, nb * KOUT], FP32, tag=f"sqi{nb}", name="sqi")
        nc.scalar.activation(
            bass.AP(sqi[:].tensor, sqi[:].offset,
                    [list(sqi[:].ap[0]), [KOUT, nb], [1, KOUT]]),
            bass.AP(a.tensor, a.offset + KOUT,
                    [list(a.ap[0]), [NFFT, nb], [1, KOUT]]),
            AF.Square)
        msum = mpool.tile([128, nb * KOUT], FP32, tag=f"ms{nb}", name="ms")
        for loc in range(nb):
            nc.gpsimd.tensor_add(msum[:, loc * KOUT:(loc + 1) * KOUT],
                                 sqr[:, loc * KOUT:(loc + 1) * KOUT],
                                 sqi[:, loc * KOUT:(loc + 1) * KOUT])
        nc.scalar.activation(gl[:, fb0 * KOUT:(fb0 + nb) * KOUT],
                             msum[:], AF.Ln, bias=epsb[:])
        for loc, fb in enumerate(fbs):
            bn6 = spool.tile([128, 6], FP32, tag="bn6", name="bn6")
            nc.vector.bn_stats(bn6[:], gl[:, fb * KOUT:(fb + 1) * KOUT])
            nc.vector.bn_aggr(mv3[:, fb, :], bn6[:])

    def stage_back(b, gl, mv, fb_lo=0, fb_hi=4):
        """rden = 1/(sqrt(var)+ceps) on DVE only (int bit-trick sqrt seed +
        two Heron steps, 5e-7 rel; keeps ACT on one table set), then
        normalize and DMA out, for blocks fb_lo..fb_hi of one item."""
        w = fb_hi - fb_lo + 1
        var = bass.AP(mv[:].tensor, mv[:].offset + 2 * fb_lo + 1,
                      [list(mv[:].ap[0]), [2, w]])
        sh = spool.tile([128, w], mybir.dt.int32, tag="sh", name="sh")
        nc.vector.tensor_scalar(sh[:], var.bitcast(mybir.dt.int32), 1, None,
                                op0=mybir.AluOpType.arith_shift_right)
        s0i = spool.tile([128, w], mybir.dt.int32, tag="s0i", name="s0i")
        nc.vector.tensor_scalar(s0i[:], sh[:], 0x1FBD1DF5, None,
                                op0=mybir.AluOpType.add)
        s_cur = s0i[:].bitcast(FP32)
        for it in range(2):
            hr = spool.tile([128, w], FP32, tag=f"hr{it}", name=f"hr{it}")
            nc.vector.reciprocal(hr[:], s_cur)
            ht = spool.tile([128, w], FP32, tag=f"ht{it}", name=f"ht{it}")
            nc.vector.tensor_mul(ht[:], var, hr[:])
            hs = spool.tile([128, w], FP32, tag=f"hs{it}", name=f"hs{it}")
            nc.vector.tensor_add(hs[:], s_cur, ht[:])
            hh = spool.tile([128, w], FP32, tag=f"hh{it}", name=f"hh{it}")
            nc.vector.tensor_scalar_mul(hh[:], hs[:], 0.5)
            s_cur = hh[:]
        uu = spool.tile([128, w], FP32, tag="uu", name="uu")
        nc.vector.tensor_scalar(uu[:], s_cur, 1.0, CEPS,
                                op0=mybir.AluOpType.mult,
                                op1=mybir.AluOpType.add)
        rden = spool.tile([128, w], FP32, tag="rden", name="rden")
        nc.vector.reciprocal(rden[:], uu[:])

        # normalize into one tile so blocks 0..3 leave in a single strided
        # DMA; block 4 (only 89 new frames) goes separately
        gn4 = None
        for fb in range(fb_lo, fb_hi + 1):
            gls = gl[:, fb * KOUT:(fb + 1) * KOUT]
            if fb < 4:
                if gn4 is None:
                    gn4 = gnpool.tile([128, 4 * KOUT], FP32, tag="gn4",
                                      name="gn4")
                gdst = gn4[:, fb * KOUT:(fb + 1) * KOUT]
            else:
                gdst = gnpool.tile([128, KOUT], FP32, tag="gn", name="gn")
            nc.vector.tensor_scalar(gdst, gls,
                                    mv[:, 2 * fb:2 * fb + 1],
                                    rden[:, fb - fb_lo:fb - fb_lo + 1],
                                    op0=mybir.AluOpType.subtract,
                                    op1=mybir.AluOpType.mult)
            if fb == 4:
                # frames 473..511 were already written by block 3
                nc.sync.dma_start(outh.ap()[b, 512:601, :], gdst[39:128, :])
        if gn4 is not None:
            nblk = min(fb_hi, 3) - fb_lo + 1
            dst = bass.AP(outh, b * F * KOUT + fb_lo * 128 * KOUT,
                          [[KOUT, 128], [128 * KOUT, nblk], [1, KOUT]])
            nc.sync.dma_start(
                dst, gn4[:, fb_lo * KOUT:(fb_lo + nblk) * KOUT].rearrange(
                    "p (f k) -> p f k", k=KOUT))

    def body():
        # issue every input DMA first: otherwise item b+1's loads queue on
        # the SP HWDGE ring behind item b's dep-gated output DMAs.
        ftiles = [fpool.tile([128, 5 * NFFT], FP32, tag=f"ftile{b}",
                             name=f"ftile{b}") for b in range(BPC)]

        def fdma(b, fb):
            srcb = AP(xh, b * T + SHIFT * F0S[fb], [[SHIFT, 128], [1, NFFT]])
            nc.sync.dma_start(ftiles[b][:, fb * NFFT:(fb + 1) * NFFT], srcb)

        fdma(0, 0)
        if reps == 1:
            nc.sync.dma_start(ident[:], idh.ap())
            nc.sync.dma_start(ident32[:], id32h.ap())
            nc.sync.dma_start(c_sb[:].rearrange("p (c k) -> p c k", k=KOUT),
                              wrh.ap().rearrange("(c p) k -> p c k", p=128))
            nc.sync.dma_start(s_sb[:].rearrange("p (c k) -> p c k", k=KOUT),
                              wih.ap().rearrange("(c p) k -> p c k", p=128))
        fdma(0, 1)
        for b in range(BPC):
            for fb in range(5):
                if (b, fb) not in ((0, 0), (0, 1)):
                    fdma(b, fb)

        pending = None
        for b in range(BPC):
            last = b == BPC - 1
            gl = glpool.tile([128, 5 * KOUT], FP32, tag="gl", name="gl")
            mv = spool.tile([128, 10], FP32, tag="mv", name="mv")
            mv3 = mv[:].rearrange("p (f two) -> p f two", two=2)
            for fbs in ((0, 1), (2, 3), (4,)):
                nb = len(fbs)
                pp = prip if nb == 2 else prip1
                pri = pp.tile([128, nb * NFFT], FP32, tag=f"pri{nb}",
                              name="pri")
                sqr = mpool.tile([128, nb * KOUT], FP32, tag=f"sqr{nb}",
                                 name="sqr")
                for loc, fb in enumerate(fbs):
                    stage_front(b, ftiles[b], fb, gl, mv3, pri, loc, sqr)
                stage_mid(b, fbs, gl, mv3, pri, sqr)
                if fbs[0] == 2 and pending is not None:
                    stage_back(*pending)
                    pending = None
                if last and fbs[0] == 2:
                    # split the final item's normalization so only block 4's
                    # short chain sits in the kernel tail
                    stage_back(b, gl, mv, 0, 3)
            if not last:
                pending = (b, gl, mv)
            else:
                stage_back(b, gl, mv, 4, 4)

    if reps == 1:
        body()
    else:
        nc.sync.dma_start(ident[:], idh.ap())
        nc.sync.dma_start(ident32[:], id32h.ap())
        nc.sync.dma_start(c_sb[:].rearrange("p (c k) -> p c k", k=KOUT),
                          wrh.ap().rearrange("(c p) k -> p c k", p=128))
        nc.sync.dma_start(s_sb[:].rearrange("p (c k) -> p c k", k=KOUT),
                          wih.ap().rearrange("(c p) k -> p c k", p=128))
        with tc.For_i(0, reps, 1):
            body()


def build_nc(mm_dt=MM_DT, reps: int = 1):
    nc = bacc.Bacc("TRN2", target_bir_lowering=False, debug=False)
    xh = nc.dram_tensor("x", [BPC, T], FP32, kind="ExternalInput")
    wrh = nc.dram_tensor("wr", [KOUT, KOUT], mm_dt, kind="ExternalInput")
    wih = nc.dram_tensor("wi", [KOUT, KOUT], mm_dt, kind="ExternalInput")
    idh = nc.dram_tensor("ident", [128, 128], mm_dt, kind="ExternalInput")
    id32h = nc.dram_tensor("ident32", [128, 128], FP32, kind="ExternalInput")
    outh = nc.dram_tensor("out", [BPC, F, KOUT], FP32, kind="ExternalOutput")
    with tile.TileContext(nc) as tc, ExitStack() as ctx:
        _build(ctx, tc, xh, wrh, wih, idh, id32h, outh, mm_dt, reps)
    nc.compile()
    return nc


def make_in_maps(x, W_real, W_imag):
    xs = np.asarray(x, dtype=np.float32).reshape(B_FULL, T)
    Wr = np.asarray(W_real, np.float32)
    Wi = np.asarray(W_imag, np.float32)
    # folded bases, rows j=1..256; j=256 halved (cos) / zero (sin, exact)
    wr_dev = np.zeros((KOUT, KOUT), np.float32)
    wi_dev = np.zeros((KOUT, KOUT), np.float32)
    wr_dev[:255] = Wr[:KOUT, 1:256].T
    wr_dev[255] = 0.5 * Wr[:KOUT, 256]
    wi_dev[:255] = Wi[:KOUT, 1:256].T
    wi_dev[255] = 0.0
    hdt = np.float16 if MM_DT == FP16 else np.float32
    ident = np.eye(128, dtype=hdt)
    return [
        {"x": np.ascontiguousarray(xs[i * BPC:(i + 1) * BPC]),
         "wr": wr_dev.astype(hdt), "wi": wi_dev.astype(hdt), "ident": ident,
         "ident32": np.eye(128, dtype=np.float32)}
        for i in range(N_CORES)
    ]


_NC_CACHE = {}


def kernel(x, W_real, W_imag):
    key = (str(MM_DT), 1)
    if key not in _NC_CACHE:
        _NC_CACHE[key] = build_nc(MM_DT, 1)
    nc = _NC_CACHE[key]
    in_maps = make_in_maps(x, W_real, W_imag)
    res = run_bass_kernel_spmd(nc, in_maps, core_ids=list(range(N_CORES)))
    out = np.concatenate([np.asarray(r["out"]) for r in res.results], axis=0)
    out = np.ascontiguousarray(out.transpose(0, 2, 1))             # [32, K, F]
    return out.reshape(B_FULL, C_FULL, KOUT, F).astype(np.float32)
